# revision 1
# baseline (speedup 1.0000x reference)
"""Trainium2 Bass kernel for nn_CustomRetrieverModel (retrieval_knn).

Late-interaction retriever scoring:
  sim4d = l2n(q_tok) @ l2n(d_tok * punct).T  -> max over doc tokens
  -> valid-weighted mean over query tokens -> avg_sim (B, M)
  logits = shuffle(avg_sim) * shuffle(Wq) * exp(log_inv_t)
  with Wq from L2-normalized CLS vectors: (center - min cand)/2.

Sharding: data-parallel over the M (document) axis. Each of the 8 cores
scores all B=32 queries against M/8 = 8 docs; q_tok/q_cls replicated,
host concatenates the per-core (B, 8) logits and applies the even/odd
column shuffle (a pure output permutation commutes with the elementwise
finale).

Device-side plan (per core):
  - q^T, d^T built via PE transposes (fp32 DMA transpose unsupported).
    The d-side transpose streams diag(mask/||d||) instead of identity,
    fusing punctuation/pad masking + L2 normalization into the
    transpose matmul for free.
  - q is NOT normalized on device: max over doc tokens commutes with the
    positive row scale 1/||q||, which is folded into the weighted-sum
    matmul weights (q_valid/||q||) instead.
  - main matmul: (2048x768) @ (768x2048) in float32r (full-rate fp32 PE
    path), accumulated over 6 K-chunks into PSUM; DVE reduce_max over
    each doc's 256 columns; per-q-chunk weighted-sum matmul accumulates
    the (32, 8) sum_sim directly in PSUM.
  - pad d tokens are zeroed (not -1e-9-masked): only changes the max
    when every real token sims below -1e-9, an O(1e-9) absolute effect.
"""

import sys

for _p in ("/opt/trn_rl_repo",):
    if _p not in sys.path:
        sys.path.append(_p)

import math

import numpy as np

import concourse.bass as bass
import concourse.tile as tile
from concourse import bacc, mybir
import concourse.bass_utils as bass_utils

# ---- problem shape (hardcoded per spec) ----
B, LQ, M, LD, H, L = 32, 64, 64, 256, 768, 3
NCORES = 8
MLOC = M // NCORES          # 8 docs per core
BQ = B * LQ                 # 2048 query rows
DR = MLOC * LD              # 2048 doc-token rows per core
KC = H // 128               # 6 contraction chunks
QT = BQ // 128              # 16 q row tiles
DT = DR // 128              # 16 d row tiles

EPS_NORM = 1e-12
EPS_DIV = 1e-10
LN2 = math.log(2.0)

F32 = mybir.dt.float32
I32 = mybir.dt.int32
U8 = mybir.dt.uint8

# ---- tuning flags ----
MM_DT = mybir.dt.float32r   # main-matmul operand dtype view
DIAG_TRANSPOSE = False      # transpose mode requires a permutation rhs; use
                            # a regular matmul against diag(scale) instead
COPY_ENG = "vector"         # engine for PSUM->SBUF transpose copies


def _emit(nc, tc, io):
    q_r = io["q_r"].ap()          # (2048, 768) f32   replicated
    d_s = io["d_s"].ap()          # (2048, 768) f32   doc shard rows
    qids = io["qids"].ap()        # (32, 64)   i32    replicated
    qcls = io["qcls"].ap()        # (32, 768)  f32    q_cls[-1] replicated
    dcls = io["dcls"].ap()        # (24, 768)  f32    d_cls shard (l*8+m, h)
    logt = io["logt"].ap()        # (32, 1)    f32    log_inv_t replicated
    out = io["out"].ap()          # (32, 8)    f32

    AF = mybir.ActivationFunctionType
    ALU = mybir.AluOpType

    if COPY_ENG == "vector":
        def _copy(out_ap, in_ap):
            nc.vector.tensor_copy(out_ap, in_ap)
    else:
        def _copy(out_ap, in_ap):
            nc.scalar.copy(out_ap, in_ap)

    class _CopyEng:
        copy = staticmethod(_copy)

    copy_eng = _CopyEng


    import contextlib
    ctx = contextlib.ExitStack()
    singles = ctx.enter_context(tc.tile_pool(name="singles", bufs=1))
    smalls = ctx.enter_context(tc.tile_pool(name="smalls", bufs=4))
    clsbig = ctx.enter_context(tc.tile_pool(name="clsbig", bufs=1))

    # ---------- constants & masks ----------
    ident = singles.tile([128, 128], F32)
    nc.vector.memset(ident, 1.0)
    nc.gpsimd.affine_select(
        out=ident, in_=ident, pattern=[[-1, 128]], base=0,
        channel_multiplier=1, compare_op=ALU.is_equal, fill=0.0,
    )

    # q_ids in per-tile layout (host pre-transposed): tile[p, c] = ids[c*128+p]
    qid_t = singles.tile([128, QT], I32)
    nc.gpsimd.dma_start(qid_t, io["qids_t"].ap())
    qv = singles.tile([128, QT], F32)       # 1.0 where q_ids != 0
    nc.vector.tensor_scalar(qv, qid_t, 0.0, None, op0=ALU.is_equal)
    nc.vector.tensor_scalar(qv, qv, -1.0, 1.0, op0=ALU.mult, op1=ALU.add)

    # n_valid from the natural (32, 64) layout: 64 - sum(q_ids == 0)
    qid_n = smalls.tile([32, 64], I32)
    nc.gpsimd.dma_start(qid_n, qids)
    qv_n = smalls.tile([32, 64], F32)
    nc.vector.tensor_scalar(qv_n, qid_n, 0.0, None, op0=ALU.is_equal)
    nv_eq = smalls.tile([32, 1], F32)
    nc.vector.reduce_sum(nv_eq, qv_n, axis=mybir.AxisListType.X)
    n_valid = smalls.tile([32, 1], F32)     # 64 - sum(eq) + eps
    nc.vector.tensor_scalar(n_valid, nv_eq, -1.0, 64.0 + EPS_DIV, op0=ALU.mult, op1=ALU.add)
    rnv = smalls.tile([32, 1], F32)
    nc.vector.reciprocal(rnv, n_valid)

    # d masks in per-tile layout
    did_t = singles.tile([128, DT], I32)
    nc.gpsimd.dma_start(did_t, io["dids_t"].ap())
    dpun_t = singles.tile([128, DT], U8)
    nc.gpsimd.dma_start(dpun_t, io["dpun_t"].ap())
    dmask = singles.tile([128, DT], F32)    # punct & (d_ids != 0)
    nc.vector.tensor_scalar(dmask, did_t, 0.0, None, op0=ALU.is_equal)
    nc.vector.tensor_scalar(dmask, dmask, -1.0, 1.0, op0=ALU.mult, op1=ALU.add)
    dpun_f = singles.tile([128, DT], F32)
    nc.vector.tensor_copy(dpun_f, dpun_t)
    nc.vector.tensor_mul(dmask, dmask, dpun_f)

    lt_t = smalls.tile([32, 1], F32)
    nc.gpsimd.dma_start(lt_t, logt)

    # ---------- rsqrt-with-clamp helper (ACT sqrt + Newton polish) ----------
    def rsqrt_clamped(dst, ss, pool):
        """dst = 1 / max(sqrt(ss), EPS_NORM), elementwise; fp32-accurate."""
        shape = list(ss.shape)
        n0 = pool.tile(shape, F32, tag="rsq_n0")
        nc.scalar.sqrt(n0, ss)
        nc.vector.tensor_scalar_max(n0, n0, 1e-30)
        r0 = pool.tile(shape, F32, tag="rsq_r0")
        nc.vector.reciprocal(r0, n0)
        t = pool.tile(shape, F32, tag="rsq_t")
        nc.vector.tensor_mul(t, ss, r0)
        nc.vector.tensor_add(t, t, n0)
        nc.vector.tensor_scalar(t, t, 0.5, EPS_NORM, op0=ALU.mult, op1=ALU.max)
        nc.vector.reciprocal(dst, t)

    qss = singles.tile([128, QT], F32)
    dss = singles.tile([128, DT], F32)

    # big SBUF transposed operands (written rounded-to-f32r by the copies)
    qT = singles.tile([128, KC, BQ], MM_DT)
    dT = singles.tile([128, KC, DR], MM_DT)

    with tc.tile_pool(name="rows", bufs=18) as rows, \
         tc.tile_pool(name="sqscr", bufs=3) as sqscr, \
         tc.tile_pool(name="tp", bufs=4, space="PSUM") as tp_pool, \
         tc.tile_pool(name="mm", bufs=3, space="PSUM") as mm_pool, \
         tc.tile_pool(name="ws", bufs=1, space="PSUM") as ws_pool, \
         tc.tile_pool(name="maxs", bufs=QT) as maxs_pool:

        # ---------- prep groups interleaved with main-matmul blocks ----------
        # emission order == per-engine program order, so main (qc, cg) blocks
        # are emitted as soon as q-group qc//4 and d-group cg exist; they fill
        # the PE while the next group's ACT/DVE/DMA work completes.
        maxs_tiles = [None] * QT

        def q_group(g):
            qtiles = []
            for j in range(4):
                c = 4 * g + j
                qt_ = rows.tile([128, H], F32, tag="rowtile")
                nc.sync.dma_start(qt_, q_r[c * 128:(c + 1) * 128, :])
                qtiles.append(qt_)
            for k in range(KC):
                tp = tp_pool.tile([128, 512], F32, tag="tp")
                for j in range(4):
                    nc.tensor.transpose(
                        tp[:, j * 128:(j + 1) * 128],
                        qtiles[j][:, k * 128:(k + 1) * 128], ident)
                nc.scalar.copy(qT[:, k, g * 512:(g + 1) * 512], tp)
            for j in range(4):
                c = 4 * g + j
                scr = sqscr.tile([128, H], F32, tag="sq")
                nc.scalar.activation(scr, qtiles[j], AF.Square,
                                     accum_out=qss[:, c:c + 1])

        def d_prep(g):
            dtiles = []
            for j in range(4):
                c = 4 * g + j
                dt_ = rows.tile([128, H], F32, tag="rowtile")
                nc.gpsimd.dma_start(dt_, d_s[c * 128:(c + 1) * 128, :])
                scr = sqscr.tile([128, H], F32, tag="sq")
                nc.scalar.activation(scr, dt_, AF.Square, accum_out=dss[:, c:c + 1])
                dtiles.append(dt_)
            # ds = dmask / max(||d||, eps); row-scale on DVE (2x mode)
            sl = slice(4 * g, 4 * g + 4)
            ds4 = smalls.tile([128, 4], F32, tag="ds4")
            rsqrt_clamped(ds4, dss[:, sl], smalls)
            nc.vector.tensor_mul(ds4, ds4, dmask[:, sl])
            for j in range(4):
                # in-place row scale (the sumsq read already ordered before)
                nc.vector.tensor_scalar(dtiles[j], dtiles[j], ds4[:, j:j + 1],
                                        None, op0=ALU.mult)
            return dtiles

        def d_tr(g, dtiles):
            for k in range(KC):
                tp = tp_pool.tile([128, 512], F32, tag="tp")
                for j in range(4):
                    nc.tensor.transpose(
                        tp[:, j * 128:(j + 1) * 128],
                        dtiles[j][:, k * 128:(k + 1) * 128], ident)
                nc.vector.tensor_copy(dT[:, k, g * 512:(g + 1) * 512], tp)

        def main_block(qc, cg):
            if maxs_tiles[qc] is None:
                maxs_tiles[qc] = maxs_pool.tile([128, MLOC], F32, tag="maxs", name=f"maxs{qc}")
            ps = mm_pool.tile([128, 512], F32, tag="mm")
            lo = cg * 512
            for k in range(KC):
                nc.tensor.matmul(ps, qT[:, k, qc * 128:(qc + 1) * 128],
                                 dT[:, k, lo:lo + 512],
                                 start=(k == 0), stop=(k == KC - 1))
            nc.vector.reduce_max(
                maxs_tiles[qc][:, 2 * cg:2 * cg + 2],
                ps[:].rearrange("p (d l) -> p d l", l=LD),
                axis=mybir.AxisListType.X)

        def cls_block():
            qc2 = clsbig.tile([32, H], F32, tag="qc2")
            nc.sync.dma_start(qc2, qcls)
            dcf = clsbig.tile([24, H], F32, tag="dcf")
            nc.sync.dma_start(dcf, dcls)

            qcss = smalls.tile([32, 1], F32, tag="qcss")
            scr1 = clsbig.tile([32, H], F32, tag="clsscr")
            nc.scalar.activation(scr1, qc2, AF.Square, accum_out=qcss)
            dcss = smalls.tile([24, 1], F32, tag="dcss")
            scr2 = clsbig.tile([24, H], F32, tag="clsscr24")
            nc.scalar.activation(scr2, dcf, AF.Square, accum_out=dcss)

            rqc = smalls.tile([32, 1], F32, tag="rqc")
            rsqrt_clamped(rqc, qcss, smalls)
            rdc = smalls.tile([24, 1], F32, tag="rdc")
            rsqrt_clamped(rdc, dcss, smalls)
            nc.scalar.mul(qc2, qc2, rqc)
            nc.scalar.mul(dcf, dcf, rdc)

            qcT = clsbig.tile([128, KC, 32], F32, tag="qcT")
            dcT = clsbig.tile([128, KC, 24], F32, tag="dcT")
            for k in range(KC):
                t1 = tp_pool.tile([128, 32], F32, tag="tp")
                nc.tensor.transpose(t1, qc2[:, k * 128:(k + 1) * 128], ident[0:32, 0:32])
                copy_eng.copy(qcT[:, k, :], t1)
                t2 = tp_pool.tile([128, 24], F32, tag="tp")
                nc.tensor.transpose(t2, dcf[:, k * 128:(k + 1) * 128], ident[0:24, 0:24])
                copy_eng.copy(dcT[:, k, :], t2)

            cls_ps = mm_pool.tile([32, 24], F32, tag="mm")
            for k in range(KC):
                nc.tensor.matmul(cls_ps, qcT[:, k, :], dcT[:, k, :],
                                 start=(k == 0), stop=(k == KC - 1))

            cls_sb = smalls.tile([32, 24], F32, tag="cls_sb")
            nc.scalar.copy(cls_sb, cls_ps)
            mind = smalls.tile([32, 8], F32, tag="mind")
            nc.vector.tensor_tensor(mind, cls_sb[:, 0:8], cls_sb[:, 8:16], op=ALU.min)
            wq2 = smalls.tile([32, 8], F32, tag="wq2")  # center - min_doc
            nc.vector.tensor_sub(wq2, cls_sb[:, 16:24], mind)
            return wq2


        W = singles.tile([128, QT, 2 * QT], F32)  # block-diagonal (128, 16, 32)
        nc.gpsimd.memset(W, 0.0)
        ws_ps = ws_pool.tile([32, MLOC], F32)   # sum_sim accumulator

        def build_W():
            # q norm -> weighted-sum weights: q_valid / ||q||
            rq = smalls.tile([128, QT], F32, name="rq")
            rsqrt_clamped(rq, qss, smalls)
            wqw = smalls.tile([128, QT], F32, name="wqw")
            nc.vector.tensor_mul(wqw, qv, rq)
            for c in range(QT):
                nc.gpsimd.tensor_copy(W[0:64, c, 2 * c:2 * c + 1], wqw[0:64, c:c + 1])
                nc.gpsimd.tensor_copy(W[64:128, c, 2 * c + 1:2 * c + 2],
                                      wqw[64:128, c:c + 1])

        emitted = set()
        ws_count = [0]

        def emit_main(qc, cg):
            emitted.add((qc, cg))
            main_block(qc, cg)

        dtiles = d_prep(0)
        for g in range(4):
            q_group(g)
            # new-row blocks need only q-group g + existing d-groups: they
            # fill the PE while d-group g's prep chain completes
            for qc in range(4 * g, 4 * g + 4):
                for cg in range(g):
                    emit_main(qc, cg)
            d_tr(g, dtiles)
            if g < 3:
                dtiles = d_prep(g + 1)   # next group's ACT/DVE chain leads
            else:
                build_W()
            for qc in range(4 * (g + 1)):
                if (qc, g) not in emitted:
                    emit_main(qc, g)
                if g == 3:
                    nc.tensor.matmul(ws_ps, W[:, qc, :], maxs_tiles[qc],
                                     start=(qc == 0), stop=(qc == QT - 1))
            if g == 1:
                wq2 = cls_block()


        # ---------- finale ----------
        it_half = smalls.tile([32, 1], F32, tag="ith")  # exp(log_inv_t)/2
        bln2 = smalls.tile([32, 1], F32, tag="bln2")
        nc.vector.memset(bln2, -LN2)
        nc.scalar.activation(it_half, lt_t, AF.Exp, bias=bln2, scale=1.0)

        avg = smalls.tile([32, 8], F32, tag="avg")
        nc.vector.tensor_scalar(avg, ws_ps, rnv, None, op0=ALU.mult)
        nc.vector.tensor_mul(avg, avg, wq2)
        outt = smalls.tile([32, 8], F32, tag="outt")
        nc.vector.tensor_scalar(outt, avg, it_half, None, op0=ALU.mult)
        nc.sync.dma_start(out, outt)

    ctx.close()


_CACHE = {}


def _build():
    if "nc" in _CACHE:
        return _CACHE["nc"]
    nc = bacc.Bacc("TRN2", target_bir_lowering=False, debug=False,
                   num_devices=NCORES)
    io = {
        "q_r": nc.dram_tensor("q_r", [BQ, H], F32, kind="ExternalInput"),
        "d_s": nc.dram_tensor("d_s", [DR, H], F32, kind="ExternalInput"),
        "qids": nc.dram_tensor("qids", [B, LQ], I32, kind="ExternalInput"),
        "qids_t": nc.dram_tensor("qids_t", [128, QT], I32, kind="ExternalInput"),
        "dids_t": nc.dram_tensor("dids_t", [128, DT], I32, kind="ExternalInput"),
        "dpun_t": nc.dram_tensor("dpun_t", [128, DT], U8, kind="ExternalInput"),
        "qcls": nc.dram_tensor("qcls", [B, H], F32, kind="ExternalInput"),
        "dcls": nc.dram_tensor("dcls", [L * MLOC, H], F32, kind="ExternalInput"),
        "logt": nc.dram_tensor("logt", [B, 1], F32, kind="ExternalInput"),
        "out": nc.dram_tensor("out", [B, MLOC], F32, kind="ExternalOutput"),
    }
    with tile.TileContext(nc) as tc:
        _emit(nc, tc, io)
    nc.compile()
    _CACHE["nc"] = nc
    return nc


def make_in_maps(q_tok, d_tok, q_cls, d_cls, log_inv_t, q_ids, d_ids,
                 d_punct_mask):
    q_r = np.ascontiguousarray(np.asarray(q_tok, np.float32).reshape(BQ, H))
    qids = np.ascontiguousarray(np.asarray(q_ids, np.int32))
    qcls = np.ascontiguousarray(np.asarray(q_cls, np.float32)[-1])
    logt = np.full((B, 1), np.float32(np.asarray(log_inv_t)), np.float32)
    qids_t = np.ascontiguousarray(qids.reshape(QT, 128).T)
    d_tok = np.asarray(d_tok, np.float32)
    d_cls = np.asarray(d_cls, np.float32)
    d_ids = np.asarray(d_ids, np.int32)
    d_pun = np.asarray(d_punct_mask).astype(np.uint8)
    in_maps = []
    for c in range(NCORES):
        sl = slice(c * MLOC, (c + 1) * MLOC)
        in_maps.append({
            "q_r": q_r,
            "d_s": np.ascontiguousarray(d_tok[sl].reshape(DR, H)),
            "qids": qids,
            "qids_t": qids_t,
            "dids_t": np.ascontiguousarray(d_ids[sl].reshape(DT, 128).T),
            "dpun_t": np.ascontiguousarray(d_pun[sl].reshape(DT, 128).T),
            "qcls": qcls,
            "dcls": np.ascontiguousarray(d_cls[:, sl, :].reshape(L * MLOC, H)),
            "logt": logt,
        })
    return in_maps


_PERM = np.concatenate([np.arange(0, M, 2), np.arange(1, M, 2)])


def kernel(q_tok, d_tok, q_cls, d_cls, log_inv_t, q_ids, d_ids, d_punct_mask,
           **run_kwargs):
    nc = _build()
    in_maps = make_in_maps(q_tok, d_tok, q_cls, d_cls, log_inv_t, q_ids,
                           d_ids, d_punct_mask)
    res = bass_utils.run_bass_kernel_spmd(nc, in_maps,
                                          core_ids=list(range(NCORES)),
                                          **run_kwargs)
    full = np.concatenate([res.results[c]["out"] for c in range(NCORES)],
                          axis=1)
    out = full[:, _PERM]
    if run_kwargs:
        kernel.last_results = res
    return out



# revision 7
# speedup vs baseline: 1.1353x; 1.1353x over previous
"""Trainium2 Bass kernel for nn_CustomRetrieverModel (retrieval_knn).

Late-interaction retriever scoring:
  sim4d = l2n(q_tok) @ l2n(d_tok * punct).T  -> max over doc tokens
  -> valid-weighted mean over query tokens -> avg_sim (B, M)
  logits = shuffle(avg_sim) * shuffle(Wq) * exp(log_inv_t)
  with Wq from L2-normalized CLS vectors: (center - min cand)/2.

Sharding: data-parallel over the M (document) axis. Each of the 8 cores
scores all B=32 queries against M/8 = 8 docs; q-side inputs replicated,
host concatenates the per-core (B, 8) logits and applies the even/odd
column shuffle (a pure output permutation commutes with the elementwise
finale).

Device-side plan (per core), v2:
  - Both matmul operands arrive HOST-PRE-TRANSPOSED in bf16 as
    (128, 4, 6, 512) = (h%128, col-group, h-chunk, col), so the PE does
    no layout transposes at all.  bf16 matmul streams at the same
    1 cycle/row as full-rate fp32r but with 2x cheaper weight loads and
    half the DMA/SBUF.
  - Row norms are computed from separate bf16 natural-layout copies via
    ACT square + free-dim accumulate -> (128, 16) sumsq tiles, keeping
    both the PE and the DVE out of the norm computation.
  - q is not normalized on device: max over doc tokens commutes with the
    positive row scale 1/||q||, which is folded into the weighted-sum
    matmul weights (q_valid/||q||).
  - d columns are scaled by mask/max(||d||,eps) in place on the DVE; the
    per-column scale row is broadcast across partitions with one tiny PE
    transpose + ones-outer-product matmuls per 512-column group.
  - main matmul: per (q-tile, d-group) block, 6 bf16 matmuls of
    (128x128)@(128x512) accumulate in PSUM; DVE reduce_max over each
    doc's 256 columns; weighted-sum matmul accumulates the (32, 8)
    sum_sim in PSUM at the end of the last group sweep.
  - CLS path: host-pre-transposed f32 operands, matmul first, then the
    separable row/column normalization is applied to the (32, 24) result
    (per-partition q scale; d scale broadcast via a tiny transpose +
    outer product), so no device-side transposes of the CLS matrices.
  - pad d tokens are zeroed (not -1e-9-masked): only changes the max
    when every real token sims below -1e-9, an O(1e-9) absolute effect.
"""

import sys

for _p in ("/opt/trn_rl_repo",):
    if _p not in sys.path:
        sys.path.append(_p)

import contextlib
import math

import numpy as np
import ml_dtypes

import concourse.bass as bass
import concourse.tile as tile
from concourse import bacc, mybir
import concourse.bass_utils as bass_utils

# ---- problem shape (hardcoded per spec) ----
B, LQ, M, LD, H, L = 32, 64, 64, 256, 768, 3
NCORES = 8
MLOC = M // NCORES          # 8 docs per core
BQ = B * LQ                 # 2048 query rows
DR = MLOC * LD              # 2048 doc-token rows per core
KC = H // 128               # 6 contraction chunks
QT = BQ // 128              # 16 q row tiles
DT = DR // 128              # 16 d row tiles
NG = 4                      # 512-wide column groups

EPS_NORM = 1e-12
EPS_DIV = 1e-10
LN2 = math.log(2.0)

F32 = mybir.dt.float32
BF16 = mybir.dt.bfloat16
I32 = mybir.dt.int32
U8 = mybir.dt.uint8


def _emit(nc, tc, io):
    q_t = io["q_t"].ap()          # (128, 4, 6, 512) bf16  q^T, replicated
    d_t = io["d_t"].ap()          # (128, 4, 6, 512) bf16  d^T shard
    q_n = io["q_n"].ap()          # (2048, 768) bf16  natural q (norms only)
    d_n = io["d_n"].ap()          # (2048, 768) bf16  natural d (norms only)
    qids = io["qids"].ap()        # (32, 64)   i32    replicated
    qcls = io["qcls"].ap()        # (32, 768)  f32    q_cls[-1] natural
    qclsT = io["qclsT"].ap()      # (128, 6, 32) f32  q_cls[-1]^T
    dcls = io["dcls"].ap()        # (24, 768)  f32    d_cls shard natural
    dclsT = io["dclsT"].ap()      # (128, 6, 24) f32  d_cls shard^T
    logt = io["logt"].ap()        # (32, 1)    f32    log_inv_t replicated
    out = io["out"].ap()          # (32, 8)    f32

    AF = mybir.ActivationFunctionType
    ALU = mybir.AluOpType

    ctx = contextlib.ExitStack()
    singles = ctx.enter_context(tc.tile_pool(name="singles", bufs=1))
    smalls = ctx.enter_context(tc.tile_pool(name="smalls", bufs=4))

    # ---------- constants & masks ----------
    identf = singles.tile([128, 128], F32)
    nc.gpsimd.memset(identf, 1.0)
    nc.gpsimd.affine_select(
        out=identf, in_=identf, pattern=[[-1, 128]], base=0,
        channel_multiplier=1, compare_op=ALU.is_equal, fill=0.0,
    )
    onesb = singles.tile([1, 128], BF16)
    nc.gpsimd.memset(onesb, 1.0)
    ones32 = singles.tile([1, 32], F32)
    nc.gpsimd.memset(ones32, 1.0)

    # q_ids in per-tile layout (host pre-transposed): tile[p, c] = ids[c*128+p]
    qid_t = singles.tile([128, QT], I32)
    nc.gpsimd.dma_start(qid_t, io["qids_t"].ap())
    qv = singles.tile([128, QT], F32)       # 1.0 where q_ids != 0
    nc.vector.tensor_scalar(qv, qid_t, 0.0, None, op0=ALU.is_equal)
    nc.vector.tensor_scalar(qv, qv, -1.0, 1.0, op0=ALU.mult, op1=ALU.add)

    # n_valid from the natural (32, 64) layout: 64 - sum(q_ids == 0)
    qid_n = smalls.tile([32, 64], I32)
    nc.gpsimd.dma_start(qid_n, qids)
    qv_n = smalls.tile([32, 64], F32)
    nc.vector.tensor_scalar(qv_n, qid_n, 0.0, None, op0=ALU.is_equal)
    nv_eq = smalls.tile([32, 1], F32)
    nc.vector.reduce_sum(nv_eq, qv_n, axis=mybir.AxisListType.X)
    n_valid = smalls.tile([32, 1], F32)     # 64 - sum(eq) + eps
    nc.vector.tensor_scalar(n_valid, nv_eq, -1.0, 64.0 + EPS_DIV,
                            op0=ALU.mult, op1=ALU.add)
    rnv = smalls.tile([32, 1], F32)
    nc.vector.reciprocal(rnv, n_valid)

    # d masks in per-tile layout
    did_t = singles.tile([128, DT], I32)
    nc.gpsimd.dma_start(did_t, io["dids_t"].ap())
    dpun_t = singles.tile([128, DT], U8)
    nc.gpsimd.dma_start(dpun_t, io["dpun_t"].ap())
    dmask = singles.tile([128, DT], F32)    # punct & (d_ids != 0)
    nc.vector.tensor_scalar(dmask, did_t, 0.0, None, op0=ALU.is_equal)
    nc.vector.tensor_scalar(dmask, dmask, -1.0, 1.0, op0=ALU.mult, op1=ALU.add)
    dpun_f = singles.tile([128, DT], F32)
    nc.vector.tensor_copy(dpun_f, dpun_t)
    nc.vector.tensor_mul(dmask, dmask, dpun_f)

    lt_t = smalls.tile([32, 1], F32)
    nc.gpsimd.dma_start(lt_t, logt)

    # ---------- rsqrt-with-clamp helper (ACT sqrt + Newton polish) ----------
    def rsqrt_clamped(dst, ss, pool):
        """dst = 1 / max(sqrt(ss), EPS_NORM), elementwise; fp32-accurate."""
        shape = list(ss.shape)
        n0 = pool.tile(shape, F32, tag="rsq_n0")
        nc.scalar.sqrt(n0, ss)
        nc.vector.tensor_scalar_max(n0, n0, 1e-30)
        r0 = pool.tile(shape, F32, tag="rsq_r0")
        nc.vector.reciprocal(r0, n0)
        t = pool.tile(shape, F32, tag="rsq_t")
        nc.vector.tensor_mul(t, ss, r0)
        nc.vector.tensor_add(t, t, n0)
        nc.vector.tensor_scalar(t, t, 0.5, EPS_NORM, op0=ALU.mult, op1=ALU.max)
        nc.vector.reciprocal(dst, t)

    qss = singles.tile([128, QT], F32)
    dss = singles.tile([128, DT], F32)

    # big SBUF operands (host pre-transposed) + per-column scale rows
    qT = singles.tile([128, NG, KC, 512], BF16)
    dT = singles.tile([128, NG, KC, 512], BF16)
    S = singles.tile([128, NG, 512], BF16)
    W = singles.tile([128, QT, 2 * QT], F32)  # block-diagonal (128, 16, 32)
    nc.gpsimd.memset(W, 0.0)

    with tc.tile_pool(name="dnat", bufs=8) as dnat_pool, \
         tc.tile_pool(name="qnat", bufs=4) as qnat_pool, \
         tc.tile_pool(name="sqscr", bufs=2) as sqscr, \
         tc.tile_pool(name="clsp", bufs=1) as clsp, \
         tc.tile_pool(name="mm", bufs=4, space="PSUM") as mm_pool, \
         tc.tile_pool(name="aux", bufs=2, space="PSUM") as aux_ps, \
         tc.tile_pool(name="ws", bufs=1, space="PSUM") as ws_pool, \
         tc.tile_pool(name="maxs", bufs=QT) as maxs_pool:

        maxs_tiles = [None] * QT
        dn_tiles = {}

        def d_dma(g):
            nc.sync.dma_start(dT[:, g], d_t[:, g])
            tiles = []
            for j in range(4):
                c = 4 * g + j
                t = dnat_pool.tile([128, H], BF16, tag="dnat")
                nc.gpsimd.dma_start(t, d_n[c * 128:(c + 1) * 128, :])
                tiles.append(t)
            dn_tiles[g] = tiles

        def q_dma():
            for qg in range(NG):
                nc.sync.dma_start(qT[:, qg], q_t[:, qg])

        def d_squares(g):
            for j, t in enumerate(dn_tiles[g]):
                c = 4 * g + j
                scr = sqscr.tile([128, H], BF16, tag="sq")
                nc.scalar.activation(scr, t, AF.Square,
                                     accum_out=dss[:, c:c + 1])

        def q_nat_squares(c0, c1):
            for c in range(c0, c1):
                t = qnat_pool.tile([128, H], BF16, tag="qnat")
                nc.gpsimd.dma_start(t, q_n[c * 128:(c + 1) * 128, :])
                scr = sqscr.tile([128, H], BF16, tag="sq")
                nc.scalar.activation(scr, t, AF.Square,
                                     accum_out=qss[:, c:c + 1])

        def d_scale(g):
            # s = mask / max(||d||, eps) for this group's 4 tiles
            sl = slice(4 * g, 4 * g + 4)
            s4 = smalls.tile([128, 4], F32, tag="s4")
            rsqrt_clamped(s4, dss[:, sl], smalls)
            nc.vector.tensor_mul(s4, s4, dmask[:, sl])
            # relayout (128, 4) -> scale row (1, 512) -> broadcast (128, 512)
            st = smalls.tile([1, 512], BF16, tag="st")
            for c in range(4):
                stp = aux_ps.tile([1, 128], F32, tag="aux")
                nc.tensor.transpose(stp, s4[:, c:c + 1], identf)
                nc.vector.tensor_copy(st[0:1, c * 128:(c + 1) * 128], stp)
            bc = aux_ps.tile([128, 512], F32, tag="aux")
            nc.tensor.matmul(bc, onesb, st, start=True, stop=True)
            nc.scalar.copy(S[:, g], bc)
            # in-place column scale of this group's dT chunks
            for k in range(KC):
                nc.vector.tensor_mul(dT[:, g, k], dT[:, g, k], S[:, g])

        def main_block(qc, g):
            if maxs_tiles[qc] is None:
                maxs_tiles[qc] = maxs_pool.tile([128, MLOC], F32, tag="maxs",
                                                name=f"maxs{qc}")
            qg, sub = divmod(qc, 4)
            ps = mm_pool.tile([128, 512], F32, tag="mm")
            for k in range(KC):
                nc.tensor.matmul(ps, qT[:, qg, k, sub * 128:(sub + 1) * 128],
                                 dT[:, g, k],
                                 start=(k == 0), stop=(k == KC - 1))
            nc.vector.reduce_max(
                maxs_tiles[qc][:, 2 * g:2 * g + 2],
                ps[:].rearrange("p (d l) -> p d l", l=LD),
                axis=mybir.AxisListType.X)

        def build_W():
            rq = smalls.tile([128, QT], F32, name="rq")
            rsqrt_clamped(rq, qss, smalls)
            wqw = smalls.tile([128, QT], F32, name="wqw")
            nc.vector.tensor_mul(wqw, qv, rq)
            for c in range(QT):
                nc.gpsimd.tensor_copy(W[0:64, c, 2 * c:2 * c + 1],
                                      wqw[0:64, c:c + 1])
                nc.gpsimd.tensor_copy(W[64:128, c, 2 * c + 1:2 * c + 2],
                                      wqw[64:128, c:c + 1])

        def cls_block():
            qcn = clsp.tile([32, H], F32, tag="qcn")
            nc.gpsimd.dma_start(qcn, qcls)
            dcn = clsp.tile([24, H], F32, tag="dcn")
            nc.gpsimd.dma_start(dcn, dcls)
            qcT = clsp.tile([128, KC, 32], F32, tag="qcT")
            nc.sync.dma_start(qcT, qclsT)
            dcT = clsp.tile([128, KC, 24], F32, tag="dcT")
            nc.sync.dma_start(dcT, dclsT)

            qcss = smalls.tile([32, 1], F32, tag="qcss")
            scr1 = clsp.tile([32, H], F32, tag="clsscr")
            nc.scalar.activation(scr1, qcn, AF.Square, accum_out=qcss)
            dcss = smalls.tile([24, 1], F32, tag="dcss")
            scr2 = clsp.tile([24, H], F32, tag="clsscr24")
            nc.scalar.activation(scr2, dcn, AF.Square, accum_out=dcss)
            rqc = smalls.tile([32, 1], F32, tag="rqc")
            rsqrt_clamped(rqc, qcss, smalls)
            rdc = smalls.tile([24, 1], F32, tag="rdc")
            rsqrt_clamped(rdc, dcss, smalls)

            # raw (32, 24) = qcT.T @ dcT, normalized afterwards (separable)
            cp = aux_ps.tile([32, 24], F32, tag="aux")
            for k in range(KC):
                nc.tensor.matmul(cp, qcT[:, k], dcT[:, k],
                                 start=(k == 0), stop=(k == KC - 1))
            raw = smalls.tile([32, 24], F32, tag="raw")
            nc.scalar.copy(raw, cp)
            nc.vector.tensor_scalar(raw, raw, rqc, None, op0=ALU.mult)
            # rdc (24,1) -> row (1,24) -> broadcast (32,24)
            rtp = aux_ps.tile([1, 24], F32, tag="aux")
            nc.tensor.transpose(rtp, rdc, identf[0:24, 0:24])
            rdT = smalls.tile([1, 24], F32, tag="rdT")
            nc.vector.tensor_copy(rdT, rtp)
            bcp = aux_ps.tile([32, 24], F32, tag="aux")
            nc.tensor.matmul(bcp, ones32, rdT, start=True, stop=True)
            rdB = smalls.tile([32, 24], F32, tag="rdB")
            nc.scalar.copy(rdB, bcp)
            nc.vector.tensor_mul(raw, raw, rdB)

            mind = smalls.tile([32, 8], F32, tag="mind")
            nc.vector.tensor_tensor(mind, raw[:, 0:8], raw[:, 8:16],
                                    op=ALU.min)
            wq2 = smalls.tile([32, 8], F32, tag="wq2")  # center - min_doc
            nc.vector.tensor_sub(wq2, raw[:, 16:24], mind)
            return wq2

        ws_ps = ws_pool.tile([32, MLOC], F32)   # sum_sim accumulator

        # ---------- schedule ----------
        d_dma(0)
        q_dma()
        d_dma(1)
        d_squares(0)
        d_scale(0)

        wq2 = None
        for g in range(NG):
            for qc in range(QT):
                main_block(qc, g)
                if g == 3:
                    nc.tensor.matmul(ws_ps, W[:, qc, :], maxs_tiles[qc],
                                     start=(qc == 0), stop=(qc == QT - 1))
                if g == 0:
                    if qc == 1:
                        q_nat_squares(0, 8)
                    elif qc == 7:
                        d_dma(2)
                        d_squares(1)
                    elif qc == 9:
                        d_scale(1)
                    elif qc == 11:
                        q_nat_squares(8, QT)
                elif g == 1:
                    if qc == 1:
                        d_dma(3)
                    elif qc == 5:
                        d_squares(2)
                    elif qc == 7:
                        d_scale(2)
                    elif qc == 10:
                        wq2 = cls_block()
                elif g == 2:
                    if qc == 5:
                        d_squares(3)
                    elif qc == 7:
                        d_scale(3)
            if g == 0:
                build_W()

        # ---------- finale ----------
        it_half = smalls.tile([32, 1], F32, tag="ith")  # exp(log_inv_t)/2
        bln2 = smalls.tile([32, 1], F32, tag="bln2")
        nc.vector.memset(bln2, -LN2)
        nc.scalar.activation(it_half, lt_t, AF.Exp, bias=bln2, scale=1.0)

        avg = smalls.tile([32, 8], F32, tag="avg")
        nc.vector.tensor_scalar(avg, ws_ps, rnv, None, op0=ALU.mult)
        nc.vector.tensor_mul(avg, avg, wq2)
        outt = smalls.tile([32, 8], F32, tag="outt")
        nc.vector.tensor_scalar(outt, avg, it_half, None, op0=ALU.mult)
        nc.sync.dma_start(out, outt)

    ctx.close()


_CACHE = {}


def _build():
    if "nc" in _CACHE:
        return _CACHE["nc"]
    nc = bacc.Bacc("TRN2", target_bir_lowering=False, debug=False,
                   num_devices=NCORES)
    io = {
        "q_t": nc.dram_tensor("q_t", [128, NG, KC, 512], BF16,
                              kind="ExternalInput"),
        "d_t": nc.dram_tensor("d_t", [128, NG, KC, 512], BF16,
                              kind="ExternalInput"),
        "q_n": nc.dram_tensor("q_n", [BQ, H], BF16, kind="ExternalInput"),
        "d_n": nc.dram_tensor("d_n", [DR, H], BF16, kind="ExternalInput"),
        "qids": nc.dram_tensor("qids", [B, LQ], I32, kind="ExternalInput"),
        "qids_t": nc.dram_tensor("qids_t", [128, QT], I32,
                                 kind="ExternalInput"),
        "dids_t": nc.dram_tensor("dids_t", [128, DT], I32,
                                 kind="ExternalInput"),
        "dpun_t": nc.dram_tensor("dpun_t", [128, DT], U8,
                                 kind="ExternalInput"),
        "qcls": nc.dram_tensor("qcls", [B, H], F32, kind="ExternalInput"),
        "qclsT": nc.dram_tensor("qclsT", [128, KC, B], F32,
                                kind="ExternalInput"),
        "dcls": nc.dram_tensor("dcls", [L * MLOC, H], F32,
                               kind="ExternalInput"),
        "dclsT": nc.dram_tensor("dclsT", [128, KC, L * MLOC], F32,
                                kind="ExternalInput"),
        "logt": nc.dram_tensor("logt", [B, 1], F32, kind="ExternalInput"),
        "out": nc.dram_tensor("out", [B, MLOC], F32, kind="ExternalOutput"),
    }
    with tile.TileContext(nc) as tc:
        _emit(nc, tc, io)
    nc.compile()
    _CACHE["nc"] = nc
    return nc


BF16NP = ml_dtypes.bfloat16


def _to_groups(x2d):
    """(2048, 768) -> (128, 4, 6, 512) with [p, g, k, j] = x[g*512+j, k*128+p]."""
    return np.ascontiguousarray(
        x2d.reshape(NG, 512, KC, 128).transpose(3, 0, 2, 1))


def make_in_maps(q_tok, d_tok, q_cls, d_cls, log_inv_t, q_ids, d_ids,
                 d_punct_mask):
    qf = np.asarray(q_tok, np.float32).reshape(BQ, H)
    q_nb = qf.astype(BF16NP)
    q_tb = _to_groups(q_nb)
    qids = np.ascontiguousarray(np.asarray(q_ids, np.int32))
    qids_t = np.ascontiguousarray(qids.reshape(QT, 128).T)
    qcls = np.ascontiguousarray(np.asarray(q_cls, np.float32)[-1])
    qclsT = np.ascontiguousarray(qcls.reshape(B, KC, 128).transpose(2, 1, 0))
    logt = np.full((B, 1), np.float32(np.asarray(log_inv_t)), np.float32)
    d_tok = np.asarray(d_tok, np.float32)
    d_cls = np.asarray(d_cls, np.float32)
    d_ids = np.asarray(d_ids, np.int32)
    d_pun = np.asarray(d_punct_mask).astype(np.uint8)
    in_maps = []
    for c in range(NCORES):
        sl = slice(c * MLOC, (c + 1) * MLOC)
        d_nb = np.ascontiguousarray(
            d_tok[sl].reshape(DR, H)).astype(BF16NP)
        dcls_c = np.ascontiguousarray(d_cls[:, sl, :].reshape(L * MLOC, H))
        in_maps.append({
            "q_t": q_tb,
            "d_t": _to_groups(d_nb),
            "q_n": np.ascontiguousarray(q_nb),
            "d_n": np.ascontiguousarray(d_nb),
            "qids": qids,
            "qids_t": qids_t,
            "dids_t": np.ascontiguousarray(d_ids[sl].reshape(DT, 128).T),
            "dpun_t": np.ascontiguousarray(d_pun[sl].reshape(DT, 128).T),
            "qcls": qcls,
            "qclsT": qclsT,
            "dcls": dcls_c,
            "dclsT": np.ascontiguousarray(
                dcls_c.reshape(L * MLOC, KC, 128).transpose(2, 1, 0)),
            "logt": logt,
        })
    return in_maps


_PERM = np.concatenate([np.arange(0, M, 2), np.arange(1, M, 2)])


def kernel(q_tok, d_tok, q_cls, d_cls, log_inv_t, q_ids, d_ids, d_punct_mask,
           **run_kwargs):
    nc = _build()
    in_maps = make_in_maps(q_tok, d_tok, q_cls, d_cls, log_inv_t, q_ids,
                           d_ids, d_punct_mask)
    res = bass_utils.run_bass_kernel_spmd(nc, in_maps,
                                          core_ids=list(range(NCORES)),
                                          **run_kwargs)
    full = np.concatenate([res.results[c]["out"] for c in range(NCORES)],
                          axis=1)
    out = full[:, _PERM]
    if run_kwargs:
        kernel.last_results = res
    return out
